# revision 1
# baseline (speedup 1.0000x reference)
# Trainium2 Bass kernel for nn_AttentionBlock (local 7x11 windowed attention).
#
# Strategy (data-parallel over batch, 4 batches/core on 8 cores):
#   - Rows are permuted to w-major order (n' = w*8 + h) so that the 7x11 local
#     attention window becomes band-structured over contiguous 128-key chunks.
#   - Per key-chunk kc (128 keys = 16 grid columns), only queries within +-5
#     grid columns can attend: a contiguous q-window of 168/208 entries.
#   - scores^T[k, q] computed directly (K=32 matmuls, 4 heads row-packed via
#     tile_position) so softmax-normalization/attn@v need NO transposes:
#       exp on ScalarE (scale folded), binary-mask multiply on GPSIMD,
#       per-(head,q) sums via ones-vector matmuls (col-packed M=1),
#       attn@v via col-packed M=32 matmuls accumulating over key chunks,
#       1/sums broadcast built with a gather-matrix matmul, applied on DVE.
#   - All matmuls run in bf16 (f32 accumulate). float32r would be more
#     accurate at the same speed, but its LDW expansion cannot carry even one
#     semaphore wait through this walrus, so it is unusable under Tile.
#   - 1/sums is computed as exp(-ln(sums)) on ScalarE: Ln+Exp share one ACT
#     table set (no table thrashing), custom-DVE recip ops don't compile here,
#     and nc.vector.reciprocal is ~6 cycles/element.
#   - b_proj is added on the host (it is zeros in this problem's setup).
import numpy as np
import ml_dtypes

B, H, WG, C, HEADS = 32, 8, 64, 256, 8
HK, WK = 7, 11
N = H * WG              # 512
HD = C // HEADS         # 32
SCALE = float(HD) ** -0.5
NCORES = 8
BPC = B // NCORES       # 4
WT = 16                 # key-chunk width (grid cols)
NKC = WG // WT          # 4
HALO = WK // 2          # 5

# n' = w*8 + h  ->  n = h*64 + w
PERM = np.array([(i % H) * WG + (i // H) for i in range(N)], dtype=np.int64)


def _kc_qwin(kc):
    c0 = max(0, WT * kc - HALO)
    c1 = min(WG, WT * kc + WT + HALO)
    return c0 * H, c1 * H


_NC_CACHE = {}

# walrus codegen rejects instructions whose sync-wait list exceeds the ISA
# struct's slot count (observed: Matmult >2 and f32r-Matmult/Ldweights >1
# fail with "Too many sync wait commands"). Tile does not split waits, so
# move the excess onto same-engine NoOps placed just before the instruction
# (FIFO order preserves the happens-before guarantee).
_WAIT_CAPS = {
    k: 1
    for k in (
        "InstMatmult", "InstLdweights", "InstActivation", "InstTensorTensor",
        "InstTensorCopy", "InstDMACopy", "InstDrain", "InstCustomDveAnt",
        "InstTensorScalarPtr", "InstMemset", "InstTensorReduce",
    )
}
_NOP_WAIT_CAP = 1


def _split_waits(nc):
    import concourse.mybir as mybir

    ctr = [0]
    for fn in nc.m.functions:
        for bb in fn.blocks:
            out = []
            for ins in bb.instructions:
                cap = _WAIT_CAPS.get(ins.__class__.__name__)
                si = getattr(ins, "sync_info", None)
                waits = list(si.on_wait) if si is not None else []
                if cap is not None and len(waits) > cap:
                    excess = waits[:-cap] if cap else waits
                    keep = waits[-cap:] if cap else []
                    while excess:
                        chunk = excess[:_NOP_WAIT_CAP]
                        excess = excess[_NOP_WAIT_CAP:]
                        w = mybir.InstEventSemaphore(
                            name=f"wsplit{ctr[0]}", ins=[], outs=[]
                        )
                        ctr[0] += 1
                        w.engine = ins.engine
                        w.sync_info = mybir.SyncInfo(
                            on_wait=chunk, on_update=[]
                        )
                        out.append(w)
                    ins.sync_info = mybir.SyncInfo(
                        on_wait=keep, on_update=list(si.on_update)
                    )
                out.append(ins)
            bb.instructions = out


def _build_nc(split_waits=True):
    key = ("nc", split_waits)
    if key in _NC_CACHE:
        return _NC_CACHE[key]
    import concourse.bass as bass
    import concourse.mybir as mybir
    import concourse.tile as tile

    f32 = mybir.dt.float32
    f32r = mybir.dt.float32r
    bf16 = mybir.dt.bfloat16
    EXP = mybir.ActivationFunctionType.Exp

    nc = bass.Bass("TRN2")

    xT = nc.dram_tensor("xT", [BPC, 2, 128, N], bf16, kind="ExternalInput")
    wqkT = nc.dram_tensor("wqkT", [2, 128, 512], bf16, kind="ExternalInput")
    wvT = nc.dram_tensor("wvT", [2, 128, 256], bf16, kind="ExternalInput")
    wpT = nc.dram_tensor("wpT", [2, 128, 256], bf16, kind="ExternalInput")
    m01T = {}
    for kc in range(NKC):
        qw0, qw1 = _kc_qwin(kc)
        m01T[kc] = nc.dram_tensor(
            f"m01T{kc}", [128, 4 * (qw1 - qw0)], bf16, kind="ExternalInput"
        )
    gsel = nc.dram_tensor("gsel", [128, 128], bf16, kind="ExternalInput")
    sumrow = nc.dram_tensor("sumrow", [1, 128], bf16, kind="ExternalInput")
    zrow = nc.dram_tensor("zrow", [1, 128], bf16, kind="ExternalInput")
    onesr = nc.dram_tensor("onesr", [1, 512], bf16, kind="ExternalInput")
    onesc = nc.dram_tensor("onesc", [128, 1], bf16, kind="ExternalInput")
    out = nc.dram_tensor("out", [BPC, N, C], f32, kind="ExternalOutput")

    with tile.TileContext(nc) as tc:
        import contextlib

        with contextlib.ExitStack() as ctx:
            singles = ctx.enter_context(tc.tile_pool(name="singles", bufs=1))
            sb = ctx.enter_context(tc.tile_pool(name="sb", bufs=2))
            ps = ctx.enter_context(tc.tile_pool(name="ps", bufs=2, space="PSUM"))

            # ---- load constants ----
            s_wqk = [singles.tile([128, 512], bf16, name=f"s_wqk{i}") for i in range(2)]
            s_wv = [singles.tile([128, 256], bf16, name=f"s_wv{i}") for i in range(2)]
            s_wp = [singles.tile([128, 256], bf16, name=f"s_wp{i}") for i in range(2)]
            for cc in range(2):
                nc.sync.dma_start(out=s_wqk[cc], in_=wqkT[cc])
                nc.sync.dma_start(out=s_wv[cc], in_=wvT[cc])
                nc.sync.dma_start(out=s_wp[cc], in_=wpT[cc])
            s_m01 = {}
            for kc in range(NKC):
                qw0, qw1 = _kc_qwin(kc)
                s_m01[kc] = singles.tile([128, 4 * (qw1 - qw0)], bf16, name=f"s_m01_{kc}")
                nc.sync.dma_start(out=s_m01[kc], in_=m01T[kc][:, :])
            s_gsel = singles.tile([128, 128], bf16)
            nc.sync.dma_start(out=s_gsel, in_=gsel[:, :])
            s_sumrow = singles.tile([1, 128], bf16)
            nc.sync.dma_start(out=s_sumrow, in_=sumrow[:, :])
            s_zrow = singles.tile([1, 128], bf16)
            nc.sync.dma_start(out=s_zrow, in_=zrow[:, :])
            s_onesr = singles.tile([1, 512], bf16)
            nc.sync.dma_start(out=s_onesr, in_=onesr[:, :])
            s_onesc = singles.tile([128, 1], bf16)
            nc.sync.dma_start(out=s_onesc, in_=onesc[:, :])

            for b in range(BPC):
                # ---- load xT (c-major) ----
                x_t = [sb.tile([128, N], bf16, tag="xT", bufs=4, name=f"x_t{i}") for i in range(2)]
                for cc in range(2):
                    nc.sync.dma_start(out=x_t[cc], in_=xT[b, cc])

                # ---- qk projection: qkT[f, n] for f in 0..512 (q: h0-7, k: h0-7)
                # psum layout: two [128,1024] tiles: fc pairs (0,1)=q, (2,3)=k
                s_qk = sb.tile([128, 2048], bf16, tag="qk", bufs=2)
                for pair in range(2):
                    p_qk = ps.tile([128, 1024], f32, tag="s", bufs=2)
                    for sub in range(2):
                        fc = pair * 2 + sub
                        for cc in range(2):
                            nc.tensor.matmul(
                                p_qk[:, sub * 512:(sub + 1) * 512],
                                lhsT=s_wqk[cc][:, fc * 128:(fc + 1) * 128],
                                rhs=x_t[cc][:, :],
                                start=(cc == 0),
                                stop=(cc == 1),
                            )
                    nc.any.tensor_copy(
                        s_qk[:, pair * 1024:(pair + 1) * 1024], p_qk[:, :]
                    )

                # ---- v projection: v[n, c] natural, bf16, per key-chunk tile
                s_v = []
                for kcb in range(NKC):
                    p_v = ps.tile([128, 1024], f32, tag="s", bufs=2)
                    for cc in range(2):
                        nc.tensor.matmul(
                            p_v[:, 0:256],
                            lhsT=x_t[cc][:, kcb * 128:(kcb + 1) * 128],
                            rhs=s_wv[cc][:, :],
                            start=(cc == 0),
                            stop=(cc == 1),
                        )
                    sv = sb.tile([128, 256], bf16, tag="v", bufs=8)
                    nc.any.tensor_copy(sv, p_v[:, 0:256])
                    s_v.append(sv)

                # ---- preclear accumulators ----
                # avT: [128 (4h x 32d), 512 q] per half; sums: rows {0,32,64,96}
                p_avT = []
                p_sums = []
                for half in range(2):
                    pa = ps.tile([128, 512], f32, tag="avT", bufs=2)
                    nc.tensor.matmul(
                        pa[:, :], lhsT=s_zrow[:, :], rhs=s_onesr[:, :],
                        start=True, stop=True, skip_group_check=True,
                    )
                    p_avT.append(pa)
                    pss = ps.tile([128, 512], f32, tag="sums", bufs=2)
                    nc.tensor.matmul(
                        pss[:, :], lhsT=s_sumrow[:, :], rhs=s_onesr[:, :],
                        start=True, stop=True, skip_group_check=True,
                    )
                    p_sums.append(pss)

                # ---- attention over key chunks ----
                # Concurrent row-tiled matmuls writing the same PSUM bank
                # crash the device, so scores go in 2-head groups with each
                # head's output slice filling a whole bank (512 f32).
                for kc in range(NKC):
                    qw0, qw1 = _kc_qwin(kc)
                    Wq = qw1 - qw0
                    for g in range(4):          # head group: heads 2g, 2g+1
                        half = g // 2
                        p_s = ps.tile([128, 1024], f32, tag="s", bufs=2)
                        for i in range(2):
                            h = 2 * g + i
                            j = h % 4           # row band within the f-chunk
                            koff = (2 + half) * 512 + kc * 128
                            nc.tensor.matmul(
                                p_s[:, i * 512: i * 512 + Wq],
                                lhsT=s_qk[32 * j:32 * j + 32, koff:koff + 128],
                                rhs=s_qk[32 * j:32 * j + 32,
                                         half * 512 + qw0: half * 512 + qw1],
                                start=True, stop=True,
                                tile_position=(32 * j, 0),
                            )
                        # exp (scale folded), PSUM->SBUF bf16
                        e_t = sb.tile([128, 2 * Wq], bf16, tag="eT", bufs=4)
                        nc.scalar.activation(
                            e_t.rearrange("p (j s) -> p j s", j=2),
                            p_s.rearrange("p (j s) -> p j s", j=2)[:, :, :Wq],
                            EXP, scale=SCALE,
                        )
                        # binary mask multiply, balanced GPSIMD/DVE
                        p_t = sb.tile([128, 2 * Wq], bf16, tag="pT", bufs=4)
                        meng = nc.gpsimd if (g % 2 == 0) else nc.vector
                        meng.tensor_mul(p_t, e_t, s_m01[kc][:, :2 * Wq])
                        # per-(head, q) sums: ones-matmul, col-packed M=1
                        for i in range(2):
                            h = 2 * g + i
                            j = h % 4
                            nc.tensor.matmul(
                                p_sums[half][32 * j:32 * j + 1, qw0:qw1],
                                lhsT=s_onesc[:, :],
                                rhs=p_t[:, i * Wq:(i + 1) * Wq],
                                start=False, stop=(kc == NKC - 1),
                                tile_position=(0, 32 * j),
                                skip_group_check=True,
                            )
                        # attn @ v: col-packed M=32, accumulate over kc
                        for i in range(2):
                            h = 2 * g + i
                            j = h % 4
                            nc.tensor.matmul(
                                p_avT[half][32 * j:32 * j + 32, qw0:qw1],
                                lhsT=s_v[kc][:, h * 32:(h + 1) * 32],
                                rhs=p_t[:, i * Wq:(i + 1) * Wq],
                                start=False, stop=(kc == NKC - 1),
                                tile_position=(0, 32 * j),
                                skip_group_check=True,
                            )

                # ---- normalize: avT_n = avT * (1/sums) broadcast over d ----
                avT_sb = []
                for half in range(2):
                    # 1/s = exp(-ln(s)); Ln and Exp share one ACT table set
                    # (custom-DVE recip doesn't compile with this walrus, and
                    # ACT Reciprocal would thrash table sets against Exp).
                    lns = sb.tile([128, 512], f32, tag="lns", bufs=2)
                    nc.scalar.activation(
                        lns, p_sums[half][:, :],
                        mybir.ActivationFunctionType.Ln,
                    )
                    r_full = sb.tile([128, 512], bf16, tag="r", bufs=2)
                    nc.scalar.activation(
                        r_full, lns, EXP, scale=-1.0,
                    )
                    p_R = ps.tile([128, 1024], f32, tag="s", bufs=2)
                    nc.tensor.matmul(
                        p_R[:, 0:512],
                        lhsT=s_gsel[:, :],
                        rhs=r_full[:, :],
                        start=True, stop=True,
                    )
                    r_sb = sb.tile([128, 512], f32, tag="Rsb", bufs=2)
                    nc.any.tensor_copy(r_sb, p_R[:, 0:512])
                    av = sb.tile([128, 512], bf16, tag="av", bufs=3)
                    nc.vector.tensor_mul(av, r_sb, p_avT[half][:, :])
                    avT_sb.append(av)

                # ---- output projection (fp32r) + store ----
                for qc in range(4):
                    p_o = ps.tile([128, 1024], f32, tag="s", bufs=2)
                    for half in range(2):
                        nc.tensor.matmul(
                            p_o[:, 0:256],
                            lhsT=avT_sb[half][:, qc * 128:(qc + 1) * 128],
                            rhs=s_wp[half][:, :],
                            start=(half == 0), stop=(half == 1),
                        )
                    o_sb = sb.tile([128, 256], f32, tag="osb", bufs=3)
                    nc.any.tensor_copy(o_sb, p_o[:, 0:256])
                    nc.sync.dma_start(
                        out=out[b, qc * 128:(qc + 1) * 128, :], in_=o_sb
                    )

    if split_waits:
        _split_waits(nc)
    _NC_CACHE[key] = nc
    return nc


def _host_inputs(x, w_qkv, mask_np):
    """Build per-core input maps (host-side reshapes/permutes only)."""
    bf16 = ml_dtypes.bfloat16
    xp = np.ascontiguousarray(x[:, PERM, :])                      # [B, N, C]
    xTp = np.ascontiguousarray(np.transpose(xp, (0, 2, 1)))       # [B, C, N]
    xTp = xTp.reshape(B, 2, 128, N).astype(bf16)

    wqkT = np.ascontiguousarray(w_qkv[:512].T).reshape(2, 128, 512).astype(bf16)
    wvT = np.ascontiguousarray(w_qkv[512:].T).reshape(2, 128, 256).astype(bf16)

    m01p = (mask_np[PERM][:, PERM] == 0.0)
    m_tiles = {}
    for kc in range(NKC):
        qw0, qw1 = _kc_qwin(kc)
        t = m01p[qw0:qw1, 128 * kc:128 * kc + 128].T.astype(np.float32)  # [128, Wq]
        m_tiles[f"m01T{kc}"] = np.ascontiguousarray(
            np.concatenate([t] * 4, axis=1)
        ).astype(bf16)

    # gather/selection matrix: out-row m takes r from row 32*(m//32)
    gs = np.zeros((128, 128), dtype=np.float32)
    for m in range(128):
        gs[32 * (m // 32), m] = 1.0
    sr = np.ones((1, 128), dtype=np.float32)
    sr[0, [0, 32, 64, 96]] = 0.0

    base = {
        "wqkT": wqkT,
        "wvT": wvT,
        "gsel": gs.astype(bf16),
        "sumrow": sr.astype(bf16),
        "zrow": np.zeros((1, 128), dtype=bf16),
        "onesr": np.ones((1, 512), dtype=bf16),
        "onesc": np.ones((128, 1), dtype=bf16),
    }
    base.update(m_tiles)
    in_maps = []
    for core in range(NCORES):
        m = dict(base)
        m["xT"] = np.ascontiguousarray(xTp[core * BPC:(core + 1) * BPC])
        in_maps.append(m)
    return in_maps


def run_sharded(x, w_qkv, w_proj, b_proj, mask, trace=False):
    """Compile+run on 8 cores; returns (out_full, BassKernelResults)."""
    from concourse.bass_utils import run_bass_kernel_spmd

    x = np.asarray(x, dtype=np.float32)
    w_qkv = np.asarray(w_qkv, dtype=np.float32)
    w_proj = np.asarray(w_proj, dtype=np.float32)
    b_proj = np.asarray(b_proj, dtype=np.float32)
    mask_np = np.asarray(mask, dtype=np.float32).reshape(N, N)

    nc = _build_nc()
    in_maps = _host_inputs(x, w_qkv, mask_np)
    import ml_dtypes as _md
    wpT = np.ascontiguousarray(w_proj.T).reshape(2, 128, 256).astype(_md.bfloat16)
    for m in in_maps:
        m["wpT"] = wpT

    res = run_bass_kernel_spmd(nc, in_maps, core_ids=list(range(NCORES)), trace=trace)

    out_full = np.empty((B, N, C), dtype=np.float32)
    for core in range(NCORES):
        od = res.results[core]["out"]          # [BPC, N, C], permuted rows
        for bi in range(BPC):
            out_full[core * BPC + bi][PERM, :] = od[bi]
    out_full += b_proj[None, None, :]
    return out_full, res


def kernel(x, w_qkv, w_proj, b_proj, mask):
    out, _ = run_sharded(x, w_qkv, w_proj, b_proj, mask, trace=False)
    return out



# revision 17
# speedup vs baseline: 1.0217x; 1.0217x over previous
# Trainium2 Bass kernel for nn_AttentionBlock (local 7x11 windowed attention).
#
# Strategy (data-parallel over batch, 4 batches/core on 8 cores):
#   - Rows are permuted to w-major order (n' = w*8 + h) so that the 7x11 local
#     attention window becomes band-structured over contiguous 128-key chunks.
#   - Per key-chunk kc (128 keys = 16 grid columns), only queries within +-5
#     grid columns can attend: a contiguous q-window, padded down to a
#     32-aligned start (the padded columns are exactly zero under the mask).
#   - scores^T[k, q] computed directly (row-packed pairs via tile_position) so
#     softmax-normalization/attn@v need NO transposes:
#       exp on ScalarE (scale folded), binary-mask multiply on DVE (2x bf16),
#       per-(q,head) sums via p_t^T @ ones matmuls with output free size 1
#       (matmul cost scales with output free size only -> sums are ~free),
#       attn@v via col-packed M=32 matmuls accumulating over key chunks.
#   - Normalizer: reciprocal on DVE over the tiny [128 q, 32 (qc,h)] sums
#     tile, one PE transpose, then 8 selection matmuls broadcast 1/sums to
#     [c, q] for the DVE normalize-multiply (fused into the PSUM->SBUF copy
#     of avT that the projection needs anyway).
#   - PSUM->SBUF copies ride on GPSIMD to keep ACT free for exp.
#   - All matmuls run in bf16 (f32 accumulate).
#   - b_proj is added on the host (it is zeros in this problem's setup).
import numpy as np
import ml_dtypes

B, H, WG, C, HEADS = 32, 8, 64, 256, 8
HK, WK = 7, 11
N = H * WG              # 512
HD = C // HEADS         # 32
SCALE = float(HD) ** -0.5
NCORES = 8
BPC = B // NCORES       # 4
WT = 16                 # key-chunk width (grid cols)
NKC = WG // WT          # 4
HALO = WK // 2          # 5

# n' = w*8 + h  ->  n = h*64 + w
PERM = np.array([(i % H) * WG + (i // H) for i in range(N)], dtype=np.int64)


def _kc_qwin(kc):
    c0 = max(0, WT * kc - HALO)
    c1 = min(WG, WT * kc + WT + HALO)
    return c0 * H, c1 * H


# Padded (32-aligned start) q-windows per key chunk. The pad columns
# [qa0, qw0) are provably masked out (|wq - wk| > HALO), so the binary mask
# zeroes them and the q-sum segments can safely read them.
QW = [_kc_qwin(kc) for kc in range(NKC)]          # real [qw0, qw1)
QA = [qw0 - (qw0 % 64) for qw0, _ in QW]          # aligned start
QE = [qw1 + (-qw1) % 64 for _, qw1 in QW]         # aligned end
WP = [QE[kc] - QA[kc] for kc in range(NKC)]       # padded width

# Sum segments per kc: uniform [sa, sa+64) so every q-sums matmul has the
# same M=64 / tile_position col in {0, 64} shape (mirrors the proven
# col-packed PSUM accumulation pattern; ragged shapes upset the device).
def _segs(kc):
    return [(a, a + 64) for a in range(QA[kc], QE[kc], 64)]


SEGS = [_segs(kc) for kc in range(NKC)]
# last kc contributing to each 128-q chunk (for matmul stop flags)
LASTKC = {}
for kc in range(NKC):
    for (sa, sb) in SEGS[kc]:
        LASTKC[sa // 128] = kc

_NC_CACHE = {}

# tile indices (of 16 per batch) whose mask-multiply runs on GPSIMD
_POOL_MULS = {0, 2, 4, 6, 8, 10, 12, 13, 14}

# walrus codegen rejects instructions whose sync-wait list exceeds the ISA
# struct's slot count (observed: Matmult >2 and f32r-Matmult/Ldweights >1
# fail with "Too many sync wait commands"). Tile does not split waits, so
# move the excess onto same-engine NoOps placed just before the instruction
# (FIFO order preserves the happens-before guarantee).
_WAIT_CAPS = {
    k: 1
    for k in (
        "InstMatmult", "InstLdweights", "InstActivation", "InstTensorTensor",
        "InstTensorCopy", "InstDMACopy", "InstDrain", "InstCustomDveAnt",
        "InstTensorScalarPtr", "InstMemset", "InstTensorReduce",
        "InstReciprocal",
    )
}
_NOP_WAIT_CAP = 1


def _split_waits(nc):
    import concourse.mybir as mybir

    ctr = [0]
    for fn in nc.m.functions:
        for bb in fn.blocks:
            out = []
            for ins in bb.instructions:
                cap = _WAIT_CAPS.get(ins.__class__.__name__)
                si = getattr(ins, "sync_info", None)
                waits = list(si.on_wait) if si is not None else []
                if cap is not None and len(waits) > cap:
                    excess = waits[:-cap] if cap else waits
                    keep = waits[-cap:] if cap else []
                    while excess:
                        chunk = excess[:_NOP_WAIT_CAP]
                        excess = excess[_NOP_WAIT_CAP:]
                        w = mybir.InstEventSemaphore(
                            name=f"wsplit{ctr[0]}", ins=[], outs=[]
                        )
                        ctr[0] += 1
                        w.engine = ins.engine
                        w.sync_info = mybir.SyncInfo(
                            on_wait=chunk, on_update=[]
                        )
                        out.append(w)
                    ins.sync_info = mybir.SyncInfo(
                        on_wait=keep, on_update=list(si.on_update)
                    )
                out.append(ins)
            bb.instructions = out


def _build_nc(split_waits=True):
    key = ("nc", split_waits)
    if key in _NC_CACHE:
        return _NC_CACHE[key]
    import concourse.bass as bass
    import concourse.mybir as mybir
    import concourse.tile as tile

    f32 = mybir.dt.float32
    bf16 = mybir.dt.bfloat16
    EXP = mybir.ActivationFunctionType.Exp

    nc = bass.Bass("TRN2")

    xT = nc.dram_tensor("xT", [BPC, 2, 128, N], bf16, kind="ExternalInput")
    wqkT = nc.dram_tensor("wqkT", [2, 128, 512], bf16, kind="ExternalInput")
    wvT = nc.dram_tensor("wvT", [2, 128, 256], bf16, kind="ExternalInput")
    wpT = nc.dram_tensor("wpT", [2, 128, 256], bf16, kind="ExternalInput")
    m01T = {}
    for kc in range(NKC):
        m01T[kc] = nc.dram_tensor(
            f"m01T{kc}", [128, 2 * WP[kc]], bf16, kind="ExternalInput"
        )
    selT = nc.dram_tensor("selT", [16, 1024], bf16, kind="ExternalInput")
    identT = nc.dram_tensor("identT", [128, 128], f32, kind="ExternalInput")
    zrow = nc.dram_tensor("zrow", [1, 128], bf16, kind="ExternalInput")
    onesr = nc.dram_tensor("onesr", [1, 512], bf16, kind="ExternalInput")
    onesc = nc.dram_tensor("onesc", [128, 1], bf16, kind="ExternalInput")
    out = nc.dram_tensor("out", [BPC, N, C], f32, kind="ExternalOutput")

    with tile.TileContext(nc) as tc:
        import contextlib

        with contextlib.ExitStack() as ctx:
            singles = ctx.enter_context(tc.tile_pool(name="singles", bufs=1))
            sb = ctx.enter_context(tc.tile_pool(name="sb", bufs=2))
            ps = ctx.enter_context(tc.tile_pool(name="ps", bufs=2, space="PSUM"))

            # ---- load constants ----
            s_wqk = [singles.tile([128, 512], bf16, name=f"s_wqk{i}") for i in range(2)]
            s_wv = [singles.tile([128, 256], bf16, name=f"s_wv{i}") for i in range(2)]
            s_wp = [singles.tile([128, 256], bf16, name=f"s_wp{i}") for i in range(2)]
            for cc in range(2):
                nc.sync.dma_start(out=s_wqk[cc], in_=wqkT[cc])
                nc.sync.dma_start(out=s_wv[cc], in_=wvT[cc])
                nc.sync.dma_start(out=s_wp[cc], in_=wpT[cc])
            s_m01 = {}
            for kc in range(NKC):
                s_m01[kc] = singles.tile([128, 2 * WP[kc]], bf16, name=f"s_m01_{kc}")
                nc.sync.dma_start(out=s_m01[kc], in_=m01T[kc][:, :])
            s_sel = singles.tile([16, 1024], bf16)
            nc.sync.dma_start(out=s_sel, in_=selT[:, :])
            s_ident = singles.tile([128, 128], f32)
            nc.sync.dma_start(out=s_ident, in_=identT[:, :])
            s_zrow = singles.tile([1, 128], bf16)
            nc.sync.dma_start(out=s_zrow, in_=zrow[:, :])
            s_onesr = singles.tile([1, 512], bf16)
            nc.sync.dma_start(out=s_onesr, in_=onesr[:, :])
            s_onesc = singles.tile([128, 1], bf16)
            nc.sync.dma_start(out=s_onesc, in_=onesc[:, :])

            # ================= cross-batch pipelined main =================
            # PE executes strictly in program order, so phases of adjacent
            # batches must be interleaved in the instruction stream: batch
            # b+1's x-load/qk/v projections are emitted piecewise between
            # batch b's attention tiles, and b's normalize+projection tail
            # overlaps b+1's first tiles via the PSUM tag rotation.

            def load_x(b):
                x_t = [
                    sb.tile([128, N], bf16, tag="xT", bufs=4, name=f"x{b}_{i}")
                    for i in range(2)
                ]
                for cc in range(2):
                    nc.sync.dma_start(out=x_t[cc], in_=xT[b, cc])
                return x_t

            def qkv_piece(b, st, piece):
                if piece == 0:
                    st["s_qk"] = sb.tile(
                        [128, 2048], bf16, tag="qk", bufs=2, name=f"qk{b}"
                    )
                if piece < 2:
                    pair = piece
                    p_qk = ps.tile([128, 1024], f32, tag="s", bufs=2)
                    for sub in range(2):
                        fc = pair * 2 + sub
                        for cc in range(2):
                            nc.tensor.matmul(
                                p_qk[:, sub * 512:(sub + 1) * 512],
                                lhsT=s_wqk[cc][:, fc * 128:(fc + 1) * 128],
                                rhs=st["x_t"][cc][:, :],
                                start=(cc == 0),
                                stop=(cc == 1),
                            )
                    nc.vector.tensor_copy(
                        st["s_qk"][:, pair * 1024:(pair + 1) * 1024], p_qk[:, :]
                    )
                else:
                    kcb = piece - 2
                    p_v = ps.tile([128, 1024], f32, tag="s", bufs=2)
                    for cc in range(2):
                        nc.tensor.matmul(
                            p_v[:, 0:256],
                            lhsT=st["x_t"][cc][:, kcb * 128:(kcb + 1) * 128],
                            rhs=s_wv[cc][:, :],
                            start=(cc == 0),
                            stop=(cc == 1),
                        )
                    sv = sb.tile(
                        [128, 256], bf16, tag="v", bufs=8, name=f"v{b}_{kcb}"
                    )
                    nc.vector.tensor_copy(sv, p_v[:, 0:256])
                    st["s_v"].append(sv)

            def preclears(b, st):
                # avT: [128 (4h x 32d), 512 q] per half; q-sums: [128 q, 32]
                st["p_avT"] = []
                for half in range(2):
                    pa = ps.tile([128, 512], f32, tag="avT", bufs=2)
                    nc.tensor.matmul(
                        pa[:, :], lhsT=s_zrow[:, :], rhs=s_onesr[:, :],
                        start=True, stop=True, skip_group_check=True,
                    )
                    st["p_avT"].append(pa)
                st["p_sums"] = ps.tile([128, 32], f32, tag="sums", bufs=1)
                nc.tensor.matmul(
                    st["p_sums"][:, :], lhsT=s_zrow[:, :], rhs=s_onesr[:, 0:32],
                    start=True, stop=True, skip_group_check=True,
                )
                st["p_rT"] = ps.tile([16, 256], f32, tag="rT", bufs=1)
                st["rT_sb"] = sb.tile(
                    [16, 256], bf16, tag="rT", bufs=2, name=f"rT{b}"
                )
                st["avT_sb"] = [
                    sb.tile([128, 512], bf16, tag="av", bufs=4, name=f"av{b}_{i}")
                    for i in range(2)
                ]

            def consume(st, kc, g, p_t):
                qw0, qw1 = QW[kc]
                qa0 = QA[kc]
                Wq = qw1 - qw0
                Wp = WP[kc]
                pad = qw0 - qa0
                half = g // 2
                for i in range(2):
                    h = 2 * g + i
                    j = h % 4
                    # attn @ v: col-packed M=32, accumulate over kc
                    nc.tensor.matmul(
                        st["p_avT"][half][32 * j:32 * j + 32, qw0:qw1],
                        lhsT=st["s_v"][kc][:, h * 32:(h + 1) * 32],
                        rhs=p_t[:, i * Wp + pad:(i * Wp) + pad + Wq],
                        start=False, stop=(kc == NKC - 1),
                        tile_position=(0, 32 * j),
                        skip_group_check=True,
                    )
                    # per-(q, head) sums: p_t^T @ ones, out free = 1
                    for (sa, sbnd) in SEGS[kc]:
                        qc = sa // 128
                        qcol = qc * 8 + h
                        nc.tensor.matmul(
                            st["p_sums"][sa % 128: sa % 128 + 64,
                                         qcol:qcol + 1],
                            lhsT=p_t[:, i * Wp + (sa - qa0):
                                     i * Wp + (sbnd - qa0)],
                            rhs=s_onesc[:, :],
                            start=False, stop=(kc == LASTKC[qc]),
                            tile_position=(0, sa % 128),
                            skip_group_check=True,
                        )

            def norm_proj(b, st, qs):
                # normalize + project q in [qs*256, qs*256+256)
                r_q = sb.tile([128, 16], f32, tag="rq", bufs=4)
                nc.vector.reciprocal(r_q, st["p_sums"][:, qs * 16:qs * 16 + 16])
                nc.tensor.matmul(
                    st["p_rT"][:, qs * 128:qs * 128 + 128], lhsT=r_q,
                    rhs=s_ident, is_transpose=True, skip_group_check=True,
                )
                nc.vector.tensor_copy(
                    st["rT_sb"][:, qs * 128:qs * 128 + 128],
                    st["p_rT"][:, qs * 128:qs * 128 + 128],
                )
                p_rb = ps.tile([128, 1024], f32, tag="s", bufs=2)
                for half in range(2):
                    for qcl in range(2):
                        qc = 2 * qs + qcl
                        idx = (qs * 2 + half) * 2 + qcl
                        nc.tensor.matmul(
                            p_rb[:, half * 512 + qc * 128:
                                 half * 512 + qc * 128 + 128],
                            lhsT=s_sel[:, idx * 128: idx * 128 + 128],
                            rhs=st["rT_sb"][:, qs * 128:qs * 128 + 128],
                            start=True, stop=True,
                        )
                rb_sb = sb.tile([128, 512], bf16, tag="rb", bufs=4)
                nc.scalar.activation(
                    rb_sb.rearrange("p (j s) -> p j s", j=2),
                    p_rb.rearrange("p (j s) -> p j s", j=2)
                        [:, :, qs * 256:qs * 256 + 256],
                    mybir.ActivationFunctionType.Copy,
                )
                for half in range(2):
                    nc.vector.tensor_mul(
                        st["avT_sb"][half][:, qs * 256:qs * 256 + 256],
                        rb_sb[:, half * 256:half * 256 + 256],
                        st["p_avT"][half][:, qs * 256:qs * 256 + 256],
                    )
                for qc in (2 * qs, 2 * qs + 1):
                    p_o = ps.tile([128, 1024], f32, tag="s", bufs=2)
                    for half in range(2):
                        nc.tensor.matmul(
                            p_o[:, 0:256],
                            lhsT=st["avT_sb"][half][:, qc * 128:(qc + 1) * 128],
                            rhs=s_wp[half][:, :],
                            start=(half == 0), stop=(half == 1),
                        )
                    o_sb = sb.tile([128, 256], f32, tag="osb", bufs=3)
                    nc.vector.tensor_copy(o_sb, p_o[:, 0:256])
                    nc.sync.dma_start(
                        out=out[b, qc * 128:(qc + 1) * 128, :], in_=o_sb
                    )

            LAG = 3
            tiles = [(kc, g) for kc in range(NKC) for g in range(4)]
            QKV_AT = {5: 0, 7: 1, 9: 2, 11: 3, 13: 4, 14: 5}

            states = {0: {"s_v": []}}
            states[0]["x_t"] = load_x(0)
            for piece in range(6):
                qkv_piece(0, states[0], piece)
            preclears(0, states[0])

            for b in range(BPC):
                st = states[b]
                nb = b + 1 if b + 1 < BPC else None
                if nb is not None:
                    states[nb] = {"s_v": []}
                pend = []
                for ti, (kc, g) in enumerate(tiles):
                    qa0 = QA[kc]
                    Wp = WP[kc]
                    half = g // 2
                    p_s = ps.tile([128, 1024], f32, tag="s", bufs=2)
                    for i in range(2):
                        h = 2 * g + i
                        j = h % 4           # row band within the f-chunk
                        koff = (2 + half) * 512 + kc * 128
                        nc.tensor.matmul(
                            p_s[:, i * 512: i * 512 + Wp],
                            lhsT=st["s_qk"][32 * j:32 * j + 32, koff:koff + 128],
                            rhs=st["s_qk"][32 * j:32 * j + 32,
                                           half * 512 + qa0:
                                           half * 512 + qa0 + Wp],
                            start=True, stop=True,
                            tile_position=(32 * j, 0),
                        )
                    # exp (scale folded), PSUM->SBUF bf16
                    e_t = sb.tile([128, 2 * Wp], bf16, tag="eT", bufs=5)
                    nc.scalar.activation(
                        e_t.rearrange("p (j s) -> p j s", j=2),
                        p_s.rearrange("p (j s) -> p j s", j=2)[:, :, :Wp],
                        EXP, scale=SCALE,
                    )
                    # binary mask multiply (bf16); ~half ride on GPSIMD to
                    # keep DVE free for the PSUM->SBUF copies
                    p_t = sb.tile([128, 2 * Wp], bf16, tag="pT", bufs=5)
                    meng = nc.gpsimd if (ti % 16) in _POOL_MULS else nc.vector
                    meng.tensor_mul(p_t, e_t, s_m01[kc][:, :2 * Wp])
                    pend.append((kc, g, p_t))
                    if nb is not None:
                        if ti == 2:
                            states[nb]["x_t"] = load_x(nb)
                        piece = QKV_AT.get(ti)
                        if piece is not None:
                            qkv_piece(nb, states[nb], piece)
                    if ti >= LAG:
                        consume(st, *pend[ti - LAG])
                        if ti - LAG == 12:
                            # q 0..255 fully accumulated: front norm + proj
                            norm_proj(b, st, 0)
                for t in pend[len(tiles) - LAG:]:
                    consume(st, *t)
                norm_proj(b, st, 1)
                if nb is not None:
                    preclears(nb, states[nb])

    if split_waits:
        _split_waits(nc)
    _NC_CACHE[key] = nc
    return nc


def _host_inputs(x, w_qkv, mask_np):
    """Build per-core input maps (host-side reshapes/permutes only)."""
    bf16 = ml_dtypes.bfloat16
    xp = np.ascontiguousarray(x[:, PERM, :])                      # [B, N, C]
    xTp = np.ascontiguousarray(np.transpose(xp, (0, 2, 1)))       # [B, C, N]
    xTp = xTp.reshape(B, 2, 128, N).astype(bf16)

    wqkT = np.ascontiguousarray(w_qkv[:512].T).reshape(2, 128, 512).astype(bf16)
    wvT = np.ascontiguousarray(w_qkv[512:].T).reshape(2, 128, 256).astype(bf16)

    m01p = (mask_np[PERM][:, PERM] == 0.0)
    m_tiles = {}
    for kc in range(NKC):
        qa0, qe1 = QA[kc], QE[kc]
        t = m01p[qa0:qe1, 128 * kc:128 * kc + 128].T.astype(np.float32)  # [128, WP]
        m_tiles[f"m01T{kc}"] = np.ascontiguousarray(
            np.concatenate([t] * 2, axis=1)
        ).astype(bf16)

    # selection matrix (per q-half): rb[c, qc*128+p] = rT[(qc%2)*8 + h(c), p]
    sel = np.zeros((16, 1024), dtype=np.float32)
    for qs in range(2):
        for half in range(2):
            for qcl in range(2):
                idx = (qs * 2 + half) * 2 + qcl
                for m in range(128):
                    sel[qcl * 8 + half * 4 + m // 32, idx * 128 + m] = 1.0

    base = {
        "wqkT": wqkT,
        "wvT": wvT,
        "selT": sel.astype(bf16),
        "identT": np.eye(128, dtype=np.float32),
        "zrow": np.zeros((1, 128), dtype=bf16),
        "onesr": np.ones((1, 512), dtype=bf16),
        "onesc": np.ones((128, 1), dtype=bf16),
    }
    base.update(m_tiles)
    in_maps = []
    for core in range(NCORES):
        m = dict(base)
        m["xT"] = np.ascontiguousarray(xTp[core * BPC:(core + 1) * BPC])
        in_maps.append(m)
    return in_maps


def run_sharded(x, w_qkv, w_proj, b_proj, mask, trace=False):
    """Compile+run on 8 cores; returns (out_full, BassKernelResults)."""
    from concourse.bass_utils import run_bass_kernel_spmd

    x = np.asarray(x, dtype=np.float32)
    w_qkv = np.asarray(w_qkv, dtype=np.float32)
    w_proj = np.asarray(w_proj, dtype=np.float32)
    b_proj = np.asarray(b_proj, dtype=np.float32)
    mask_np = np.asarray(mask, dtype=np.float32).reshape(N, N)

    nc = _build_nc()
    in_maps = _host_inputs(x, w_qkv, mask_np)
    import ml_dtypes as _md
    wpT = np.ascontiguousarray(w_proj.T).reshape(2, 128, 256).astype(_md.bfloat16)
    for m in in_maps:
        m["wpT"] = wpT

    res = run_bass_kernel_spmd(nc, in_maps, core_ids=list(range(NCORES)), trace=trace)

    out_full = np.empty((B, N, C), dtype=np.float32)
    for core in range(NCORES):
        od = res.results[core]["out"]          # [BPC, N, C], permuted rows
        for bi in range(BPC):
            out_full[core * BPC + bi][PERM, :] = od[bi]
    out_full += b_proj[None, None, :]
    return out_full, res


def kernel(x, w_qkv, w_proj, b_proj, mask):
    out, _ = run_sharded(x, w_qkv, w_proj, b_proj, mask, trace=False)
    return out


# revision 22
# speedup vs baseline: 1.3490x; 1.3203x over previous
# Trainium2 Bass kernel for nn_AttentionBlock (local 7x11 windowed attention).
#
# Strategy (data-parallel over batch, 4 batches/core on 8 cores):
#   - Rows are permuted to w-major order (n' = w*8 + h) so that the 7x11 local
#     attention window becomes band-structured over contiguous 128-key chunks.
#   - Per key-chunk kc (128 keys = 16 grid columns), only queries within +-5
#     grid columns can attend: a contiguous q-window, padded down to a
#     32-aligned start (the padded columns are exactly zero under the mask).
#   - scores^T[k, q] computed directly (row-packed pairs via tile_position) so
#     softmax-normalization/attn@v need NO transposes:
#       exp on ScalarE (scale folded), binary-mask multiply on DVE (2x bf16),
#       per-(q,head) sums via p_t^T @ ones matmuls with output free size 1
#       (matmul cost scales with output free size only -> sums are ~free),
#       attn@v via col-packed M=32 matmuls accumulating over key chunks.
#   - Normalizer: reciprocal on DVE over the tiny [128 q, 32 (qc,h)] sums
#     tile, one PE transpose, then 8 selection matmuls broadcast 1/sums to
#     [c, q] for the DVE normalize-multiply (fused into the PSUM->SBUF copy
#     of avT that the projection needs anyway).
#   - PSUM->SBUF copies ride on GPSIMD to keep ACT free for exp.
#   - All matmuls run in bf16 (f32 accumulate).
#   - b_proj is added on the host (it is zeros in this problem's setup).
import numpy as np
import ml_dtypes

B, H, WG, C, HEADS = 32, 8, 64, 256, 8
HK, WK = 7, 11
N = H * WG              # 512
HD = C // HEADS         # 32
SCALE = float(HD) ** -0.5
NCORES = 8
BPC = B // NCORES       # 4
WT = 16                 # key-chunk width (grid cols)
NKC = WG // WT          # 4
HALO = WK // 2          # 5

# n' = w*8 + h  ->  n = h*64 + w
PERM = np.array([(i % H) * WG + (i // H) for i in range(N)], dtype=np.int64)


def _kc_qwin(kc):
    c0 = max(0, WT * kc - HALO)
    c1 = min(WG, WT * kc + WT + HALO)
    return c0 * H, c1 * H


# Padded (32-aligned start) q-windows per key chunk. The pad columns
# [qa0, qw0) are provably masked out (|wq - wk| > HALO), so the binary mask
# zeroes them and the q-sum segments can safely read them.
QW = [_kc_qwin(kc) for kc in range(NKC)]          # real [qw0, qw1)
QA = [qw0 - (qw0 % 64) for qw0, _ in QW]          # aligned start
QE = [qw1 + (-qw1) % 64 for _, qw1 in QW]         # aligned end
WP = [QE[kc] - QA[kc] for kc in range(NKC)]       # padded width

# Sum segments per kc: uniform [sa, sa+64) so every q-sums matmul has the
# same M=64 / tile_position col in {0, 64} shape (mirrors the proven
# col-packed PSUM accumulation pattern; ragged shapes upset the device).
def _segs(kc):
    return [(a, a + 64) for a in range(QA[kc], QE[kc], 64)]


SEGS = [_segs(kc) for kc in range(NKC)]
# last kc contributing to each 128-q chunk (for matmul stop flags)
LASTKC = {}
for kc in range(NKC):
    for (sa, sb) in SEGS[kc]:
        LASTKC[sa // 128] = kc

# packed-constant layout (columns of constsT [128, CW] bf16)
M01OFF = []
_off = 0
for _kc in range(NKC):
    M01OFF.append(_off)
    _off += 2 * WP[_kc]
SELOFF = _off            # sel [16, 1024] at rows 0:16
ONESCOFF = SELOFF + 1024  # ones column [128, 1]
ONESROFF = ONESCOFF + 1   # ones row [1, 512] (row 0)
ZROWOFF = ONESROFF + 512  # zero row [1, 128] (row 0)
CW = ZROWOFF + 128

_NC_CACHE = {}

# tile indices (of 16 per batch) whose mask-multiply runs on GPSIMD
_POOL_MULS = {0, 2, 4, 6, 8, 10, 12, 13, 14}

# walrus codegen rejects instructions whose sync-wait list exceeds the ISA
# struct's slot count (observed: Matmult >2 and f32r-Matmult/Ldweights >1
# fail with "Too many sync wait commands"). Tile does not split waits, so
# move the excess onto same-engine NoOps placed just before the instruction
# (FIFO order preserves the happens-before guarantee).
_WAIT_CAPS = {
    k: 1
    for k in (
        "InstMatmult", "InstLdweights", "InstActivation", "InstTensorTensor",
        "InstTensorCopy", "InstDMACopy", "InstDrain", "InstCustomDveAnt",
        "InstTensorScalarPtr", "InstMemset", "InstTensorReduce",
        "InstReciprocal",
    )
}
_NOP_WAIT_CAP = 1


def _split_waits(nc):
    import concourse.mybir as mybir

    ctr = [0]
    for fn in nc.m.functions:
        for bb in fn.blocks:
            out = []
            for ins in bb.instructions:
                cap = _WAIT_CAPS.get(ins.__class__.__name__)
                si = getattr(ins, "sync_info", None)
                waits = list(si.on_wait) if si is not None else []
                if cap is not None and len(waits) > cap:
                    excess = waits[:-cap] if cap else waits
                    keep = waits[-cap:] if cap else []
                    while excess:
                        chunk = excess[:_NOP_WAIT_CAP]
                        excess = excess[_NOP_WAIT_CAP:]
                        w = mybir.InstEventSemaphore(
                            name=f"wsplit{ctr[0]}", ins=[], outs=[]
                        )
                        ctr[0] += 1
                        w.engine = ins.engine
                        w.sync_info = mybir.SyncInfo(
                            on_wait=chunk, on_update=[]
                        )
                        out.append(w)
                    ins.sync_info = mybir.SyncInfo(
                        on_wait=keep, on_update=list(si.on_update)
                    )
                out.append(ins)
            bb.instructions = out


def _build_nc(split_waits=True):
    key = ("nc", split_waits)
    if key in _NC_CACHE:
        return _NC_CACHE[key]
    import concourse.bass as bass
    import concourse.mybir as mybir
    import concourse.tile as tile

    f32 = mybir.dt.float32
    bf16 = mybir.dt.bfloat16
    EXP = mybir.ActivationFunctionType.Exp

    nc = bass.Bass("TRN2")

    xT = nc.dram_tensor("xT", [BPC, 128, 2 * N], bf16, kind="ExternalInput")
    wT = nc.dram_tensor("wT", [128, 2048], bf16, kind="ExternalInput")
    constsT = nc.dram_tensor("constsT", [128, CW], bf16, kind="ExternalInput")
    identT = nc.dram_tensor("identT", [128, 128], f32, kind="ExternalInput")
    out = nc.dram_tensor("out", [BPC, N, C], f32, kind="ExternalOutput")

    with tile.TileContext(nc) as tc:
        import contextlib

        with contextlib.ExitStack() as ctx:
            singles = ctx.enter_context(tc.tile_pool(name="singles", bufs=1))
            sb = ctx.enter_context(tc.tile_pool(name="sb", bufs=2))
            ps = ctx.enter_context(tc.tile_pool(name="ps", bufs=2, space="PSUM"))

            # ---- load constants (batched: 3 DMAs total) ----
            s_w = singles.tile([128, 2048], bf16, name="s_w")
            nc.sync.dma_start(out=s_w, in_=wT[:, :])
            s_wqk = [s_w[:, cc * 1024: cc * 1024 + 512] for cc in range(2)]
            s_wv = [s_w[:, cc * 1024 + 512: cc * 1024 + 768] for cc in range(2)]
            s_wp = [s_w[:, cc * 1024 + 768: cc * 1024 + 1024] for cc in range(2)]
            s_consts = singles.tile([128, CW], bf16, name="s_consts")
            nc.sync.dma_start(out=s_consts, in_=constsT[:, :])
            s_m01 = {
                kc: s_consts[:, M01OFF[kc]: M01OFF[kc] + 2 * WP[kc]]
                for kc in range(NKC)
            }
            s_sel = s_consts[0:16, SELOFF:SELOFF + 1024]
            s_onesc = s_consts[:, ONESCOFF:ONESCOFF + 1]
            s_onesr = s_consts[0:1, ONESROFF:ONESROFF + 512]
            s_zrow = s_consts[0:1, ZROWOFF:ZROWOFF + 128]
            s_ident = singles.tile([128, 128], f32)

            # ================= cross-batch pipelined main =================
            # PE executes strictly in program order, so phases of adjacent
            # batches must be interleaved in the instruction stream: batch
            # b+1's x-load/qk/v projections are emitted piecewise between
            # batch b's attention tiles, and b's normalize+projection tail
            # overlaps b+1's first tiles via the PSUM tag rotation.

            def load_x(b):
                xt = sb.tile([128, 2 * N], bf16, tag="xT", bufs=2, name=f"x{b}")
                nc.sync.dma_start(out=xt, in_=xT[b])
                return [xt[:, cc * N:(cc + 1) * N] for cc in range(2)]

            def qkv_piece(b, st, piece):
                if piece == 0:
                    st["s_qk"] = sb.tile(
                        [128, 2048], bf16, tag="qk", bufs=2, name=f"qk{b}"
                    )
                if piece < 2:
                    pair = piece
                    for sub in range(2):
                        fc = pair * 2 + sub
                        p_qk = ps.tile([128, 512], f32, tag="s", bufs=3)
                        for cc in range(2):
                            nc.tensor.matmul(
                                p_qk[:, :],
                                lhsT=s_wqk[cc][:, fc * 128:(fc + 1) * 128],
                                rhs=st["x_t"][cc][:, :],
                                start=(cc == 0),
                                stop=(cc == 1),
                            )
                        nc.vector.tensor_copy(
                            st["s_qk"][:, fc * 512:(fc + 1) * 512], p_qk[:, :]
                        )
                else:
                    kcb = piece - 2
                    p_v = ps.tile([128, 512], f32, tag="s", bufs=3)
                    for cc in range(2):
                        nc.tensor.matmul(
                            p_v[:, 0:256],
                            lhsT=st["x_t"][cc][:, kcb * 128:(kcb + 1) * 128],
                            rhs=s_wv[cc][:, :],
                            start=(cc == 0),
                            stop=(cc == 1),
                        )
                    sv = sb.tile(
                        [128, 256], bf16, tag="v", bufs=8, name=f"v{b}_{kcb}"
                    )
                    nc.vector.tensor_copy(sv, p_v[:, 0:256])
                    st["s_v"].append(sv)

            def preclears_avt(b, st):
                # avT: [128 (4h x 32d), 512 q] per half; bufs=4 so batch b+1
                # preclears never wait on batch b's normalize reads
                st["p_avT"] = []
                for half in range(2):
                    pa = ps.tile([128, 512], f32, tag="avT", bufs=4)
                    nc.tensor.matmul(
                        pa[:, :], lhsT=s_zrow[:, :], rhs=s_onesr[:, :],
                        start=True, stop=True, skip_group_check=True,
                    )
                    st["p_avT"].append(pa)
                st["avT_sb"] = [
                    sb.tile([128, 512], bf16, tag="av", bufs=4, name=f"av{b}_{i}")
                    for i in range(2)
                ]

            def preclears_sums(b, st):
                # one carved bank: cols 0:32 q-sums accumulator [128 q, 32],
                # cols 32:288 (rows 0:16) the transposed-reciprocal target
                st["p_nrm"] = ps.tile(
                    [128, 288], f32, tag="sums", bufs=1, name=f"pnrm{b}"
                )
                st["p_sums"] = st["p_nrm"][:, 0:32]
                st["p_rT"] = st["p_nrm"][0:16, 32:288]
                nc.tensor.matmul(
                    st["p_sums"], lhsT=s_zrow[:, :], rhs=s_onesr[:, 0:32],
                    start=True, stop=True, skip_group_check=True,
                )
                st["rT_sb"] = sb.tile(
                    [16, 256], bf16, tag="rT", bufs=2, name=f"rT{b}"
                )

            def consume(st, kc, g, p_t):
                qw0, qw1 = QW[kc]
                qa0 = QA[kc]
                Wq = qw1 - qw0
                Wp = WP[kc]
                pad = qw0 - qa0
                for i in range(2):
                    h = g + 4 * i
                    j = g
                    # attn @ v: col-packed M=32, accumulate over kc
                    nc.tensor.matmul(
                        st["p_avT"][i][32 * j:32 * j + 32, qw0:qw1],
                        lhsT=st["s_v"][kc][:, h * 32:(h + 1) * 32],
                        rhs=p_t[:, i * Wp + pad:(i * Wp) + pad + Wq],
                        start=False, stop=(kc == NKC - 1),
                        tile_position=(0, 32 * j),
                        skip_group_check=True,
                    )
                    # per-(q, head) sums: p_t^T @ ones, out free = 1
                    for (sa, sbnd) in SEGS[kc]:
                        qc = sa // 128
                        qcol = qc * 8 + h
                        nc.tensor.matmul(
                            st["p_sums"][sa % 128: sa % 128 + 64,
                                         qcol:qcol + 1],
                            lhsT=p_t[:, i * Wp + (sa - qa0):
                                     i * Wp + (sbnd - qa0)],
                            rhs=s_onesc[:, :],
                            start=False, stop=(kc == LASTKC[qc]),
                            tile_position=(0, sa % 128),
                            skip_group_check=True,
                        )

            def norm_proj(b, st, qs):
                # normalize + project q in [qs*256, qs*256+256)
                r_q = sb.tile([128, 16], f32, tag="rq", bufs=4)
                nc.vector.reciprocal(r_q, st["p_sums"][:, qs * 16:qs * 16 + 16])
                nc.tensor.matmul(
                    st["p_rT"][:, qs * 128:qs * 128 + 128], lhsT=r_q,
                    rhs=s_ident, is_transpose=True, skip_group_check=True,
                )
                nc.vector.tensor_copy(
                    st["rT_sb"][:, qs * 128:qs * 128 + 128],
                    st["p_rT"][:, qs * 128:qs * 128 + 128],
                )
                p_rb = ps.tile([128, 512], f32, tag="s", bufs=3)
                for half in range(2):
                    for qcl in range(2):
                        idx = (qs * 2 + half) * 2 + qcl
                        nc.tensor.matmul(
                            p_rb[:, half * 256 + qcl * 128:
                                 half * 256 + qcl * 128 + 128],
                            lhsT=s_sel[:, idx * 128: idx * 128 + 128],
                            rhs=st["rT_sb"][:, qs * 128:qs * 128 + 128],
                            start=True, stop=True,
                        )
                rb_sb = sb.tile([128, 512], bf16, tag="rb", bufs=4)
                nc.scalar.activation(
                    rb_sb, p_rb, mybir.ActivationFunctionType.Copy
                )
                for half in range(2):
                    nc.vector.tensor_mul(
                        st["avT_sb"][half][:, qs * 256:qs * 256 + 256],
                        rb_sb[:, half * 256:half * 256 + 256],
                        st["p_avT"][half][:, qs * 256:qs * 256 + 256],
                    )
                for qc in (2 * qs, 2 * qs + 1):
                    p_o = ps.tile([128, 512], f32, tag="s", bufs=3)
                    for half in range(2):
                        nc.tensor.matmul(
                            p_o[:, 0:256],
                            lhsT=st["avT_sb"][half][:, qc * 128:(qc + 1) * 128],
                            rhs=s_wp[half][:, :],
                            start=(half == 0), stop=(half == 1),
                        )
                    o_sb = sb.tile([128, 256], f32, tag="osb", bufs=3)
                    nc.vector.tensor_copy(o_sb, p_o[:, 0:256])
                    nc.sync.dma_start(
                        out=out[b, qc * 128:(qc + 1) * 128, :], in_=o_sb
                    )

            LAG = 3
            tiles = [(kc, g) for kc in range(NKC) for g in range(4)]
            # head pairing per tile: (g, g+4) share the PE row band 32g, so
            # the two scores matmuls are same-quadrant (serialized on HW) and
            # one [128,512] PSUM bank holds both 256-wide slices.
            QKV_AT = {5: 0, 7: 1, 9: 2, 11: 3, 13: 4, 14: 5}

            states = {0: {"s_v": []}}
            states[0]["x_t"] = load_x(0)
            nc.sync.dma_start(out=s_consts, in_=constsT[:, :])
            nc.sync.dma_start(out=s_ident, in_=identT[:, :])
            for piece in range(6):
                qkv_piece(0, states[0], piece)
            preclears_avt(0, states[0])
            preclears_sums(0, states[0])

            for b in range(BPC):
                st = states[b]
                nb = b + 1 if b + 1 < BPC else None
                if nb is not None:
                    states[nb] = {"s_v": []}
                pend = []
                for ti, (kc, g) in enumerate(tiles):
                    qa0 = QA[kc]
                    Wp = WP[kc]
                    p_s = ps.tile([128, 512], f32, tag="s", bufs=3)
                    for i in range(2):
                        h = g + 4 * i       # heads (g, g+4): same row band
                        koff = (2 + i) * 512 + kc * 128
                        nc.tensor.matmul(
                            p_s[:, i * 256: i * 256 + Wp],
                            lhsT=st["s_qk"][32 * g:32 * g + 32, koff:koff + 128],
                            rhs=st["s_qk"][32 * g:32 * g + 32,
                                           i * 512 + qa0:
                                           i * 512 + qa0 + Wp],
                            start=True, stop=True,
                            tile_position=(32 * g, 0),
                        )
                    # exp (scale folded), PSUM->SBUF bf16
                    e_t = sb.tile([128, 2 * Wp], bf16, tag="eT", bufs=5)
                    nc.scalar.activation(
                        e_t.rearrange("p (j s) -> p j s", j=2),
                        p_s.rearrange("p (j s) -> p j s", j=2)[:, :, :Wp],
                        EXP, scale=SCALE,
                    )
                    # binary mask multiply (bf16); ~half ride on GPSIMD to
                    # keep DVE free for the PSUM->SBUF copies
                    p_t = sb.tile([128, 2 * Wp], bf16, tag="pT", bufs=5)
                    meng = nc.gpsimd if (ti % 16) in _POOL_MULS else nc.vector
                    meng.tensor_mul(p_t, e_t, s_m01[kc][:, :2 * Wp])
                    pend.append((kc, g, p_t))
                    if nb is not None:
                        if ti == 2:
                            states[nb]["x_t"] = load_x(nb)
                        piece = QKV_AT.get(ti)
                        if piece is not None:
                            qkv_piece(nb, states[nb], piece)
                    if ti >= LAG:
                        consume(st, *pend[ti - LAG])
                        if ti - LAG == 11:
                            # q 0..255 fully accumulated: front norm + proj
                            norm_proj(b, st, 0)
                for t in pend[len(tiles) - LAG:]:
                    consume(st, *t)
                if nb is not None:
                    preclears_avt(nb, states[nb])
                norm_proj(b, st, 1)
                if nb is not None:
                    preclears_sums(nb, states[nb])

    if split_waits:
        _split_waits(nc)
    _NC_CACHE[key] = nc
    return nc


def _host_inputs(x, w_qkv, w_proj, mask_np):
    """Build per-core input maps (host-side reshapes/permutes only)."""
    bf16 = ml_dtypes.bfloat16
    xp = np.ascontiguousarray(x[:, PERM, :])                      # [B, N, C]
    xTp = np.ascontiguousarray(np.transpose(xp, (0, 2, 1)))       # [B, C, N]
    xTp = xTp.reshape(B, 2, 128, N).astype(bf16)

    wqkT = np.ascontiguousarray(w_qkv[:512].T).reshape(2, 128, 512)
    wvT = np.ascontiguousarray(w_qkv[512:].T).reshape(2, 128, 256)
    wpT = np.ascontiguousarray(w_proj.T).reshape(2, 128, 256)
    wcc = np.concatenate([wqkT, wvT, wpT], axis=2)                # [2,128,1024]
    wT = np.concatenate([wcc[0], wcc[1]], axis=1).astype(bf16)    # [128, 2048]

    consts = np.zeros((128, CW), dtype=np.float32)
    m01p = (mask_np[PERM][:, PERM] == 0.0)
    for kc in range(NKC):
        qa0, qe1 = QA[kc], QE[kc]
        t = m01p[qa0:qe1, 128 * kc:128 * kc + 128].T.astype(np.float32)
        consts[:, M01OFF[kc]:M01OFF[kc] + 2 * WP[kc]] = np.concatenate(
            [t] * 2, axis=1
        )
    # selection matrix (per q-half): rb[c, qc*128+p] = rT[(qc%2)*8 + h(c), p]
    for qs in range(2):
        for half in range(2):
            for qcl in range(2):
                idx = (qs * 2 + half) * 2 + qcl
                for m in range(128):
                    consts[qcl * 8 + half * 4 + m // 32,
                           SELOFF + idx * 128 + m] = 1.0
    consts[:, ONESCOFF] = 1.0
    consts[0, ONESROFF:ONESROFF + 512] = 1.0
    # ZROW region stays zero

    base = {
        "wT": wT,
        "constsT": consts.astype(bf16),
        "identT": np.eye(128, dtype=np.float32),
    }
    in_maps = []
    for core in range(NCORES):
        m = dict(base)
        xc = xTp[core * BPC:(core + 1) * BPC]                     # [BPC,2,128,N]
        m["xT"] = np.ascontiguousarray(
            np.concatenate([xc[:, 0], xc[:, 1]], axis=2)          # [BPC,128,2N]
        )
        in_maps.append(m)
    return in_maps


def run_sharded(x, w_qkv, w_proj, b_proj, mask, trace=False):
    """Compile+run on 8 cores; returns (out_full, BassKernelResults)."""
    from concourse.bass_utils import run_bass_kernel_spmd

    x = np.asarray(x, dtype=np.float32)
    w_qkv = np.asarray(w_qkv, dtype=np.float32)
    w_proj = np.asarray(w_proj, dtype=np.float32)
    b_proj = np.asarray(b_proj, dtype=np.float32)
    mask_np = np.asarray(mask, dtype=np.float32).reshape(N, N)

    nc = _build_nc()
    in_maps = _host_inputs(x, w_qkv, w_proj, mask_np)

    res = run_bass_kernel_spmd(nc, in_maps, core_ids=list(range(NCORES)), trace=trace)

    out_full = np.empty((B, N, C), dtype=np.float32)
    for core in range(NCORES):
        od = res.results[core]["out"]          # [BPC, N, C], permuted rows
        for bi in range(BPC):
            out_full[core * BPC + bi][PERM, :] = od[bi]
    out_full += b_proj[None, None, :]
    return out_full, res


def kernel(x, w_qkv, w_proj, b_proj, mask):
    out, _ = run_sharded(x, w_qkv, w_proj, b_proj, mask, trace=False)
    return out


# revision 29
# speedup vs baseline: 1.3499x; 1.0007x over previous
# Trainium2 Bass kernel for nn_AttentionBlock (local 7x11 windowed attention).
#
# Strategy (data-parallel over batch, 4 batches/core on 8 cores):
#   - Rows are permuted to w-major order (n' = w*8 + h) so that the 7x11 local
#     attention window becomes band-structured over contiguous 128-key chunks.
#   - Per key-chunk kc (128 keys = 16 grid columns), only queries within +-5
#     grid columns can attend: a contiguous q-window, padded down to a
#     32-aligned start (the padded columns are exactly zero under the mask).
#   - scores^T[k, q] computed directly (row-packed pairs via tile_position) so
#     softmax-normalization/attn@v need NO transposes:
#       exp on ScalarE (scale folded), binary-mask multiply on DVE (2x bf16),
#       per-(q,head) sums via p_t^T @ ones matmuls with output free size 1
#       (matmul cost scales with output free size only -> sums are ~free),
#       attn@v via col-packed M=32 matmuls accumulating over key chunks.
#   - Normalizer: reciprocal on DVE over the tiny [128 q, 32 (qc,h)] sums
#     tile, one PE transpose, then selection matmuls broadcast 1/sums to
#     [c, q] for the DVE normalize-multiply; done per q-half so the front
#     half overlaps the remaining key-chunks.
#   - Cross-batch software pipelining: batch b+1's x-load/qk/v projections
#     are interleaved into batch b's attention tiles (the PE is in-order,
#     so overlap must be explicit in the instruction stream). GPSIMD takes
#     ~half the mask multiplies (it cannot touch PSUM); PSUM->SBUF copies
#     split across DVE/ACT.
#   - All matmuls run in bf16 (f32 accumulate).
#   - b_proj is added on the host (it is zeros in this problem's setup).
import numpy as np
import ml_dtypes

B, H, WG, C, HEADS = 32, 8, 64, 256, 8
HK, WK = 7, 11
N = H * WG              # 512
HD = C // HEADS         # 32
SCALE = float(HD) ** -0.5
NCORES = 8
BPC = B // NCORES       # 4
WT = 16                 # key-chunk width (grid cols)
NKC = WG // WT          # 4
HALO = WK // 2          # 5

# n' = w*8 + h  ->  n = h*64 + w
PERM = np.array([(i % H) * WG + (i // H) for i in range(N)], dtype=np.int64)


def _kc_qwin(kc):
    c0 = max(0, WT * kc - HALO)
    c1 = min(WG, WT * kc + WT + HALO)
    return c0 * H, c1 * H


# Padded (32-aligned start) q-windows per key chunk. The pad columns
# [qa0, qw0) are provably masked out (|wq - wk| > HALO), so the binary mask
# zeroes them and the q-sum segments can safely read them.
QW = [_kc_qwin(kc) for kc in range(NKC)]          # real [qw0, qw1)
QA = [qw0 - (qw0 % 64) for qw0, _ in QW]          # aligned start
QE = [qw1 + (-qw1) % 64 for _, qw1 in QW]         # aligned end
WP = [QE[kc] - QA[kc] for kc in range(NKC)]       # padded width

# Sum segments per kc: uniform [sa, sa+64) so every q-sums matmul has the
# same M=64 / tile_position col in {0, 64} shape (mirrors the proven
# col-packed PSUM accumulation pattern; ragged shapes upset the device).
def _segs(kc):
    return [(a, a + 64) for a in range(QA[kc], QE[kc], 64)]


SEGS = [_segs(kc) for kc in range(NKC)]
# last kc contributing to each 128-q chunk (for matmul stop flags)
LASTKC = {}
for kc in range(NKC):
    for (sa, sb) in SEGS[kc]:
        LASTKC[sa // 128] = kc

# packed-constant layout (columns of constsT [128, CW] bf16)
M01OFF = []
_off = 0
for _kc in range(NKC):
    M01OFF.append(_off)
    _off += 2 * WP[_kc]
SELOFF = _off            # sel [16, 1024] at rows 0:16
ONESCOFF = SELOFF + 1024  # ones column [128, 1]
ONESROFF = ONESCOFF + 1   # ones row [1, 512] (row 0)
ZROWOFF = ONESROFF + 512  # zero row [1, 128] (row 0)
CW = ZROWOFF + 128

_NC_CACHE = {}

# tile indices (of 16 per batch) whose mask-multiply runs on GPSIMD
_POOL_MULS = {0, 2, 4, 6, 8, 10, 12, 13, 14}

# walrus codegen rejects instructions whose sync-wait list exceeds the ISA
# struct's slot count (observed: Matmult >2 and f32r-Matmult/Ldweights >1
# fail with "Too many sync wait commands"). Tile does not split waits, so
# move the excess onto same-engine NoOps placed just before the instruction
# (FIFO order preserves the happens-before guarantee).
_WAIT_CAPS = {
    k: 1
    for k in (
        "InstMatmult", "InstLdweights", "InstActivation", "InstTensorTensor",
        "InstTensorCopy", "InstDMACopy", "InstDrain", "InstCustomDveAnt",
        "InstTensorScalarPtr", "InstMemset", "InstTensorReduce",
        "InstReciprocal",
    )
}
_NOP_WAIT_CAP = 1


def _split_waits(nc):
    import concourse.mybir as mybir

    ctr = [0]
    for fn in nc.m.functions:
        for bb in fn.blocks:
            out = []
            for ins in bb.instructions:
                cap = _WAIT_CAPS.get(ins.__class__.__name__)
                si = getattr(ins, "sync_info", None)
                waits = list(si.on_wait) if si is not None else []
                if cap is not None and len(waits) > cap:
                    excess = waits[:-cap] if cap else waits
                    keep = waits[-cap:] if cap else []
                    while excess:
                        chunk = excess[:_NOP_WAIT_CAP]
                        excess = excess[_NOP_WAIT_CAP:]
                        w = mybir.InstEventSemaphore(
                            name=f"wsplit{ctr[0]}", ins=[], outs=[]
                        )
                        ctr[0] += 1
                        w.engine = ins.engine
                        w.sync_info = mybir.SyncInfo(
                            on_wait=chunk, on_update=[]
                        )
                        out.append(w)
                    ins.sync_info = mybir.SyncInfo(
                        on_wait=keep, on_update=list(si.on_update)
                    )
                out.append(ins)
            bb.instructions = out


def _build_nc(split_waits=True):
    key = ("nc", split_waits)
    if key in _NC_CACHE:
        return _NC_CACHE[key]
    import concourse.bass as bass
    import concourse.mybir as mybir
    import concourse.tile as tile

    f32 = mybir.dt.float32
    bf16 = mybir.dt.bfloat16
    EXP = mybir.ActivationFunctionType.Exp

    nc = bass.Bass("TRN2")

    xT = nc.dram_tensor("xT", [BPC, 128, 2 * N], bf16, kind="ExternalInput")
    wT = nc.dram_tensor("wT", [128, 2048], bf16, kind="ExternalInput")
    constsT = nc.dram_tensor("constsT", [128, CW], bf16, kind="ExternalInput")
    identT = nc.dram_tensor("identT", [128, 128], f32, kind="ExternalInput")
    out = nc.dram_tensor("out", [BPC, N, C], f32, kind="ExternalOutput")

    with tile.TileContext(nc) as tc:
        import contextlib

        with contextlib.ExitStack() as ctx:
            singles = ctx.enter_context(tc.tile_pool(name="singles", bufs=1))
            sb = ctx.enter_context(tc.tile_pool(name="sb", bufs=2))
            ps = ctx.enter_context(tc.tile_pool(name="ps", bufs=2, space="PSUM"))

            # ---- load constants (batched: 3 DMAs total) ----
            s_w = singles.tile([128, 2048], bf16, name="s_w")
            nc.sync.dma_start(out=s_w, in_=wT[:, :])
            s_wqk = [s_w[:, cc * 1024: cc * 1024 + 512] for cc in range(2)]
            s_wv = [s_w[:, cc * 1024 + 512: cc * 1024 + 768] for cc in range(2)]
            s_wp = [s_w[:, cc * 1024 + 768: cc * 1024 + 1024] for cc in range(2)]
            s_consts = singles.tile([128, CW], bf16, name="s_consts")
            nc.sync.dma_start(out=s_consts, in_=constsT[:, :])
            s_m01 = {
                kc: s_consts[:, M01OFF[kc]: M01OFF[kc] + 2 * WP[kc]]
                for kc in range(NKC)
            }
            s_sel = s_consts[0:16, SELOFF:SELOFF + 1024]
            s_onesc = s_consts[:, ONESCOFF:ONESCOFF + 1]
            s_onesr = s_consts[0:1, ONESROFF:ONESROFF + 512]
            s_zrow = s_consts[0:1, ZROWOFF:ZROWOFF + 128]
            s_ident = singles.tile([128, 128], f32)

            # ================= cross-batch pipelined main =================
            # PE executes strictly in program order, so phases of adjacent
            # batches must be interleaved in the instruction stream: batch
            # b+1's x-load/qk/v projections are emitted piecewise between
            # batch b's attention tiles, and b's normalize+projection tail
            # overlaps b+1's first tiles via the PSUM tag rotation.

            def load_x(b):
                xt = sb.tile([128, 2 * N], bf16, tag="xT", bufs=2, name=f"x{b}")
                nc.sync.dma_start(out=xt, in_=xT[b])
                return [xt[:, cc * N:(cc + 1) * N] for cc in range(2)]

            def qkv_piece(b, st, piece):
                if piece == 0:
                    st["s_qk"] = sb.tile(
                        [128, 2048], bf16, tag="qk", bufs=2, name=f"qk{b}"
                    )
                if piece < 2:
                    pair = piece
                    for sub in range(2):
                        fc = pair * 2 + sub
                        p_qk = ps.tile([128, 512], f32, tag="s", bufs=4)
                        for cc in range(2):
                            nc.tensor.matmul(
                                p_qk[:, :],
                                lhsT=s_wqk[cc][:, fc * 128:(fc + 1) * 128],
                                rhs=st["x_t"][cc][:, :],
                                start=(cc == 0),
                                stop=(cc == 1),
                            )
                        nc.vector.tensor_copy(
                            st["s_qk"][:, fc * 512:(fc + 1) * 512], p_qk[:, :]
                        )
                else:
                    kcb = piece - 2
                    p_v = ps.tile([128, 512], f32, tag="s", bufs=4)
                    for cc in range(2):
                        nc.tensor.matmul(
                            p_v[:, 0:256],
                            lhsT=st["x_t"][cc][:, kcb * 128:(kcb + 1) * 128],
                            rhs=s_wv[cc][:, :],
                            start=(cc == 0),
                            stop=(cc == 1),
                        )
                    sv = sb.tile(
                        [128, 256], bf16, tag="v", bufs=8, name=f"v{b}_{kcb}"
                    )
                    nc.vector.tensor_copy(sv, p_v[:, 0:256])
                    st["s_v"].append(sv)

            def preclears(b, st):
                # avT: [128 (4h x 32d), 512 q] per half; q-sums: [128 q, 32]
                st["p_avT"] = []
                for half in range(2):
                    pa = ps.tile([128, 512], f32, tag="avT", bufs=2)
                    nc.tensor.matmul(
                        pa[:, :], lhsT=s_zrow[:, :], rhs=s_onesr[:, :],
                        start=True, stop=True, skip_group_check=True,
                    )
                    st["p_avT"].append(pa)
                st["p_sums"] = ps.tile(
                    [128, 32], f32, tag="sums", bufs=1, name=f"psums{b}"
                )
                nc.tensor.matmul(
                    st["p_sums"][:, :], lhsT=s_zrow[:, :], rhs=s_onesr[:, 0:32],
                    start=True, stop=True, skip_group_check=True,
                )
                st["p_rT"] = ps.tile(
                    [16, 256], f32, tag="rT", bufs=1, name=f"prT{b}"
                )
                st["rT_sb"] = sb.tile(
                    [16, 256], bf16, tag="rT", bufs=2, name=f"rT{b}"
                )
                st["avT_sb"] = [
                    sb.tile([128, 512], bf16, tag="av", bufs=4, name=f"av{b}_{i}")
                    for i in range(2)
                ]

            def consume(st, kc, g, p_t):
                qw0, qw1 = QW[kc]
                qa0 = QA[kc]
                Wq = qw1 - qw0
                Wp = WP[kc]
                pad = qw0 - qa0
                for i in range(2):
                    h = g + 4 * i
                    j = g
                    # attn @ v: col-packed M=32, accumulate over kc
                    nc.tensor.matmul(
                        st["p_avT"][i][32 * j:32 * j + 32, qw0:qw1],
                        lhsT=st["s_v"][kc][:, h * 32:(h + 1) * 32],
                        rhs=p_t[:, i * Wp + pad:(i * Wp) + pad + Wq],
                        start=False, stop=(kc == NKC - 1),
                        tile_position=(0, 32 * j),
                        skip_group_check=True,
                    )
                    # per-(q, head) sums: p_t^T @ ones, out free = 1
                    for (sa, sbnd) in SEGS[kc]:
                        qc = sa // 128
                        qcol = qc * 8 + h
                        nc.tensor.matmul(
                            st["p_sums"][sa % 128: sa % 128 + 64,
                                         qcol:qcol + 1],
                            lhsT=p_t[:, i * Wp + (sa - qa0):
                                     i * Wp + (sbnd - qa0)],
                            rhs=s_onesc[:, :],
                            start=False, stop=(kc == LASTKC[qc]),
                            tile_position=(0, sa % 128),
                            skip_group_check=True,
                        )

            def norm_proj(b, st, qs):
                # normalize + project q in [qs*256, qs*256+256)
                r_q = sb.tile([128, 16], f32, tag="rq", bufs=4)
                nc.vector.reciprocal(r_q, st["p_sums"][:, qs * 16:qs * 16 + 16])
                nc.tensor.matmul(
                    st["p_rT"][:, qs * 128:qs * 128 + 128], lhsT=r_q,
                    rhs=s_ident, is_transpose=True, skip_group_check=True,
                )
                nc.vector.tensor_copy(
                    st["rT_sb"][:, qs * 128:qs * 128 + 128],
                    st["p_rT"][:, qs * 128:qs * 128 + 128],
                )
                p_rb = ps.tile([128, 512], f32, tag="s", bufs=4)
                for half in range(2):
                    for qcl in range(2):
                        idx = (qs * 2 + half) * 2 + qcl
                        nc.tensor.matmul(
                            p_rb[:, half * 256 + qcl * 128:
                                 half * 256 + qcl * 128 + 128],
                            lhsT=s_sel[:, idx * 128: idx * 128 + 128],
                            rhs=st["rT_sb"][:, qs * 128:qs * 128 + 128],
                            start=True, stop=True,
                        )
                rb_sb = sb.tile([128, 512], bf16, tag="rb", bufs=4)
                nc.scalar.activation(
                    rb_sb, p_rb, mybir.ActivationFunctionType.Copy
                )
                for half in range(2):
                    nc.vector.tensor_mul(
                        st["avT_sb"][half][:, qs * 256:qs * 256 + 256],
                        rb_sb[:, half * 256:half * 256 + 256],
                        st["p_avT"][half][:, qs * 256:qs * 256 + 256],
                    )
                for qc in (2 * qs, 2 * qs + 1):
                    p_o = ps.tile([128, 512], f32, tag="s", bufs=4)
                    for half in range(2):
                        nc.tensor.matmul(
                            p_o[:, 0:256],
                            lhsT=st["avT_sb"][half][:, qc * 128:(qc + 1) * 128],
                            rhs=s_wp[half][:, :],
                            start=(half == 0), stop=(half == 1),
                        )
                    o_sb = sb.tile([128, 256], f32, tag="osb", bufs=3)
                    nc.vector.tensor_copy(o_sb, p_o[:, 0:256])
                    nc.sync.dma_start(
                        out=out[b, qc * 128:(qc + 1) * 128, :], in_=o_sb
                    )

            LAG = 3
            tiles = [(kc, g) for kc in range(NKC) for g in range(4)]
            # head pairing per tile: (g, g+4) share the PE row band 32g, so
            # the two scores matmuls are same-quadrant (serialized on HW) and
            # one [128,512] PSUM bank holds both 256-wide slices.
            QKV_AT = {5: 0, 7: 1, 9: 2, 11: 3, 13: 4, 14: 5}

            states = {0: {"s_v": []}}
            states[0]["x_t"] = load_x(0)
            nc.sync.dma_start(out=s_ident, in_=identT[:, :])
            for piece in range(6):
                qkv_piece(0, states[0], piece)
            preclears(0, states[0])

            def tile_front(st, ti):
                kc, g = tiles[ti]
                qa0 = QA[kc]
                Wp = WP[kc]
                p_s = ps.tile([128, 512], f32, tag="s", bufs=4)
                for i in range(2):
                    koff = (2 + i) * 512 + kc * 128
                    nc.tensor.matmul(
                        p_s[:, i * 256: i * 256 + Wp],
                        lhsT=st["s_qk"][32 * g:32 * g + 32, koff:koff + 128],
                        rhs=st["s_qk"][32 * g:32 * g + 32,
                                       i * 512 + qa0:
                                       i * 512 + qa0 + Wp],
                        start=True, stop=True,
                        tile_position=(32 * g, 0),
                    )
                # exp (scale folded), PSUM->SBUF bf16
                e_t = sb.tile([128, 2 * Wp], bf16, tag="eT", bufs=5)
                nc.scalar.activation(
                    e_t.rearrange("p (j s) -> p j s", j=2),
                    p_s.rearrange("p (j s) -> p j s", j=2)[:, :, :Wp],
                    EXP, scale=SCALE,
                )
                # binary mask multiply (bf16); ~half ride on GPSIMD to
                # keep DVE free for the PSUM->SBUF copies
                p_t = sb.tile([128, 2 * Wp], bf16, tag="pT", bufs=5)
                meng = nc.gpsimd if (ti % 16) in _POOL_MULS else nc.vector
                meng.tensor_mul(p_t, e_t, s_m01[kc][:, :2 * Wp])
                st["pend"].append((kc, g, p_t))

            states[0]["pend"] = []
            for b in range(BPC):
                st = states[b]
                nb = b + 1 if b + 1 < BPC else None
                if nb is not None:
                    states[nb] = {"s_v": [], "pend": []}
                pend = st["pend"]
                for ti in range(len(pend), len(tiles)):
                    tile_front(st, ti)
                    if nb is not None:
                        if ti == 2:
                            states[nb]["x_t"] = load_x(nb)
                        piece = QKV_AT.get(ti)
                        if piece is not None:
                            qkv_piece(nb, states[nb], piece)
                    if ti >= LAG:
                        consume(st, *pend[ti - LAG])
                        if ti - LAG == 12:
                            # q 0..255 fully accumulated: front norm + proj
                            norm_proj(b, st, 0)
                for t in pend[len(tiles) - LAG:]:
                    consume(st, *t)
                norm_proj(b, st, 1)
                if nb is not None:
                    preclears(nb, states[nb])

    if split_waits:
        _split_waits(nc)
    _NC_CACHE[key] = nc
    return nc


def _host_inputs(x, w_qkv, w_proj, mask_np):
    """Build per-core input maps (host-side reshapes/permutes only)."""
    bf16 = ml_dtypes.bfloat16
    xp = np.ascontiguousarray(x[:, PERM, :])                      # [B, N, C]
    xTp = np.ascontiguousarray(np.transpose(xp, (0, 2, 1)))       # [B, C, N]
    xTp = xTp.reshape(B, 2, 128, N).astype(bf16)

    wqkT = np.ascontiguousarray(w_qkv[:512].T).reshape(2, 128, 512)
    wvT = np.ascontiguousarray(w_qkv[512:].T).reshape(2, 128, 256)
    wpT = np.ascontiguousarray(w_proj.T).reshape(2, 128, 256)
    wcc = np.concatenate([wqkT, wvT, wpT], axis=2)                # [2,128,1024]
    wT = np.concatenate([wcc[0], wcc[1]], axis=1).astype(bf16)    # [128, 2048]

    consts = np.zeros((128, CW), dtype=np.float32)
    m01p = (mask_np[PERM][:, PERM] == 0.0)
    for kc in range(NKC):
        qa0, qe1 = QA[kc], QE[kc]
        t = m01p[qa0:qe1, 128 * kc:128 * kc + 128].T.astype(np.float32)
        consts[:, M01OFF[kc]:M01OFF[kc] + 2 * WP[kc]] = np.concatenate(
            [t] * 2, axis=1
        )
    # selection matrix (per q-half): rb[c, qc*128+p] = rT[(qc%2)*8 + h(c), p]
    for qs in range(2):
        for half in range(2):
            for qcl in range(2):
                idx = (qs * 2 + half) * 2 + qcl
                for m in range(128):
                    consts[qcl * 8 + half * 4 + m // 32,
                           SELOFF + idx * 128 + m] = 1.0
    consts[:, ONESCOFF] = 1.0
    consts[0, ONESROFF:ONESROFF + 512] = 1.0
    # ZROW region stays zero

    base = {
        "wT": wT,
        "constsT": consts.astype(bf16),
        "identT": np.eye(128, dtype=np.float32),
    }
    in_maps = []
    for core in range(NCORES):
        m = dict(base)
        xc = xTp[core * BPC:(core + 1) * BPC]                     # [BPC,2,128,N]
        m["xT"] = np.ascontiguousarray(
            np.concatenate([xc[:, 0], xc[:, 1]], axis=2)          # [BPC,128,2N]
        )
        in_maps.append(m)
    return in_maps


def run_sharded(x, w_qkv, w_proj, b_proj, mask, trace=False):
    """Compile+run on 8 cores; returns (out_full, BassKernelResults)."""
    from concourse.bass_utils import run_bass_kernel_spmd

    x = np.asarray(x, dtype=np.float32)
    w_qkv = np.asarray(w_qkv, dtype=np.float32)
    w_proj = np.asarray(w_proj, dtype=np.float32)
    b_proj = np.asarray(b_proj, dtype=np.float32)
    mask_np = np.asarray(mask, dtype=np.float32).reshape(N, N)

    nc = _build_nc()
    in_maps = _host_inputs(x, w_qkv, w_proj, mask_np)

    res = run_bass_kernel_spmd(nc, in_maps, core_ids=list(range(NCORES)), trace=trace)

    out_full = np.empty((B, N, C), dtype=np.float32)
    for core in range(NCORES):
        od = res.results[core]["out"]          # [BPC, N, C], permuted rows
        for bi in range(BPC):
            out_full[core * BPC + bi][PERM, :] = od[bi]
    out_full += b_proj[None, None, :]
    return out_full, res


def kernel(x, w_qkv, w_proj, b_proj, mask):
    out, _ = run_sharded(x, w_qkv, w_proj, b_proj, mask, trace=False)
    return out
